# revision 1
# baseline (speedup 1.0000x reference)
"""BERT self-attention (B=8, S=1024, D=768, H=12) on 8 TRN2 NeuronCores.

Strategy
--------
Data-parallel over batch: core b handles batch element b (no collectives).

Per core, everything is computed in a "transposed" layout that keeps the
tensor engine's contraction dimension in the partition axis:

  1. mixedT[e, s] = sum_d W^T[d, e] * x^T[d, s] + bias[e]   (e-tile pairs of
     2 heads; psum evacuated by DVE with per-partition bias, rounding to
     fp32r for downstream matmuls)
  2. Q=K=V => the score matrix is symmetric: scores[t, s] = scores[s, t].
     The exp'd score tile in [t, s] layout therefore equals the transposed
     (unnormalized) probability matrix needed as the moving operand of the
     context matmul -- no probability transposes at all.
     scores tile  = (MIX chunk)^T @ Z_h  where Z_h zero-masks the other
     head of the pair (kills cross-head terms, keeps K=128 partition-
     aligned matmuls).
     U = exp(0.125 * scores + mask[t])  fused into the ACT psum evacuation
     (mask enters as the per-partition bias).
  3. ctx'^T[dh, s] (+ softmax denominator as row 64) accumulate over the
     eight t-chunks with stationary [xl | ones] [128, 65], moving U.
  4. PE-transpose of ctx'^T 128-column chunks gives ctx[s, dh] with the
     denominator as column 64; reciprocal + per-partition scalar multiply
     normalizes during the psum evacuation; contiguous-ish DMA to the
     output in natural [s, d] layout.
"""

import numpy as np

import concourse.bacc as bacc
import concourse.tile as tile
from concourse import mybir
from concourse.bass_utils import run_bass_kernel_spmd
from concourse.masks import make_identity

B, S, D = 8, 1024, 768
H, DH = 12, 64
NP = 6            # e-tile pairs (2 heads each)
NT = 8            # t-chunks / s-chunks of 128
F32 = mybir.dt.float32
F32R = mybir.dt.float32r
EXP = mybir.ActivationFunctionType.Exp

_CACHED_NC = None


def build_nc():
    nc = bacc.Bacc("TRN2", target_bir_lowering=False)

    xT = nc.dram_tensor("xT", [D, S], F32R, kind="ExternalInput")
    wT = nc.dram_tensor("wT", [D, D], F32R, kind="ExternalInput")
    bias_d = nc.dram_tensor("bias_d", [128, NP], F32, kind="ExternalInput")
    mask_d = nc.dram_tensor("mask_d", [128, NT], F32, kind="ExternalInput")
    out_d = nc.dram_tensor("out", [S, D], F32, kind="ExternalOutput")

    with tile.TileContext(nc) as tc:
        with (
            tc.tile_pool(name="consts", bufs=1) as consts,
            tc.tile_pool(name="big", bufs=1) as big,
            tc.tile_pool(name="zpool", bufs=4) as zpool,
            tc.tile_pool(name="upool", bufs=3) as upool,
            tc.tile_pool(name="xlnpool", bufs=4) as xlnpool,
            tc.tile_pool(name="ctpool", bufs=2) as ctpool,
            tc.tile_pool(name="outpool", bufs=4) as outpool,
            tc.tile_pool(name="rpool", bufs=4) as rpool,
            tc.tile_pool(name="ps_s", bufs=2, space="PSUM") as ps_s,
            tc.tile_pool(name="ps_c", bufs=1, space="PSUM") as ps_c,
            tc.tile_pool(name="ps_t", bufs=2, space="PSUM") as ps_t,
        ):
            ident = consts.tile([128, 128], F32)
            make_identity(nc, ident)
            bias_t = consts.tile([128, NP], F32)
            nc.sync.dma_start(out=bias_t, in_=bias_d[:, :])
            mask_t = consts.tile([128, NT], F32)
            nc.sync.dma_start(out=mask_t, in_=mask_d[:, :])

            wts = big.tile([128, NP, D], F32R)
            xts = big.tile([128, NP, S], F32R)
            for k in range(NP):
                nc.sync.dma_start(out=wts[:, k, :], in_=wT[k * 128:(k + 1) * 128, :])
                nc.sync.dma_start(out=xts[:, k, :], in_=xT[k * 128:(k + 1) * 128, :])

            mix = big.tile([128, NP, S], F32R)

            for j in range(NP):
                # ---- mixedT for head pair j ----
                pm = ps_s.tile([128, S], F32, name="psc")
                for n in range(2):
                    for k in range(NP):
                        nc.tensor.matmul(
                            pm[:, n * 512:(n + 1) * 512],
                            lhsT=wts[:, k, j * 128:(j + 1) * 128],
                            rhs=xts[:, k, n * 512:(n + 1) * 512],
                            start=(k == 0),
                            stop=(k == NP - 1),
                        )
                nc.vector.tensor_scalar_add(mix[:, j, :], pm, bias_t[:, j:j + 1])

                # ---- zero-masked copies (one per head of the pair) ----
                zs = []
                for q in range(2):
                    z = zpool.tile([128, S], F32R, name="z")
                    lo = q * 64
                    olo = (1 - q) * 64
                    nc.gpsimd.memset(z[olo:olo + 64, :].bitcast(F32), 0.0)
                    nc.vector.tensor_copy(
                        out=z[lo:lo + 64, :], in_=mix[lo:lo + 64, j, :].bitcast(F32)
                    )
                    zs.append(z)

                # ---- xl in natural layout (+ ones column) via PE transpose ----
                xlns = []
                for q in range(2):
                    xln = xlnpool.tile([128, NT, DH + 1], F32R, name="xln")
                    nc.gpsimd.memset(xln.bitcast(F32), 1.0)
                    xlns.append(xln)
                for i in range(NT):
                    pt = ps_t.tile([128, 128], F32, name="pt")
                    nc.tensor.transpose(
                        pt, mix[:, j, i * 128:(i + 1) * 128].bitcast(F32), ident
                    )
                    for q in range(2):
                        nc.vector.tensor_copy(
                            out=xlns[q][:, i, 0:DH], in_=pt[:, q * 64:q * 64 + 64]
                        )

                # ---- attention for the two heads ----
                for q in range(2):
                    h = 2 * j + q
                    pc = ps_c.tile([DH + 1, S], F32, name="pc")
                    for i in range(NT):
                        psc = ps_s.tile([128, S], F32, name="psc")
                        for n in range(2):
                            nc.tensor.matmul(
                                psc[:, n * 512:(n + 1) * 512],
                                lhsT=mix[:, j, i * 128:(i + 1) * 128],
                                rhs=zs[q][:, n * 512:(n + 1) * 512],
                                start=True,
                                stop=True,
                            )
                        u = upool.tile([128, S], F32R, name="u")
                        nc.scalar.activation(
                            out=u, in_=psc, func=EXP,
                            bias=mask_t[:, i:i + 1], scale=0.125,
                        )
                        for n in range(2):
                            nc.tensor.matmul(
                                pc[:, n * 512:(n + 1) * 512],
                                lhsT=xlns[q][:, i, :],
                                rhs=u[:, n * 512:(n + 1) * 512],
                                start=(i == 0),
                                stop=(i == NT - 1),
                            )
                    ct = ctpool.tile([DH + 1, S], F32, name="ct")
                    nc.vector.tensor_copy(out=ct, in_=pc)
                    for sj in range(NT):
                        po = ps_t.tile([128, DH + 1], F32, name="pt")
                        nc.tensor.transpose(
                            po,
                            ct[:, sj * 128:(sj + 1) * 128],
                            ident[0:DH + 1, 0:DH + 1],
                        )
                        rcol = rpool.tile([128, 1], F32, name="rcol")
                        nc.vector.reciprocal(out=rcol, in_=po[:, DH:DH + 1])
                        ot = outpool.tile([128, DH], F32, name="ot")
                        nc.vector.tensor_scalar_mul(ot, po[:, 0:DH], rcol)
                        nc.sync.dma_start(
                            out=out_d[sj * 128:(sj + 1) * 128, h * 64:(h + 1) * 64],
                            in_=ot,
                        )

    nc.compile()
    return nc


def kernel(x, attention_mask, W, b, _profile=None):
    global _CACHED_NC
    if _CACHED_NC is None:
        _CACHED_NC = build_nc()
    nc = _CACHED_NC

    x = np.asarray(x, dtype=np.float32)
    attention_mask = np.asarray(attention_mask, dtype=np.float32)
    W = np.asarray(W, dtype=np.float32)
    b = np.asarray(b, dtype=np.float32)

    wT = np.ascontiguousarray(W.T)
    bias_cols = np.ascontiguousarray(b.reshape(NP, 128).T)

    in_maps = []
    for i in range(B):
        in_maps.append({
            "xT": np.ascontiguousarray(x[i].T),
            "wT": wT,
            "bias_d": bias_cols,
            "mask_d": np.ascontiguousarray(
                attention_mask[i, 0, 0].reshape(NT, 128).T
            ),
        })

    kwargs = dict(_profile) if _profile else {}
    res = run_bass_kernel_spmd(nc, in_maps, core_ids=list(range(B)), **kwargs)
    out = np.stack([res.results[i]["out"] for i in range(B)], axis=0)
    if _profile:
        kernel.last_results = res
    return out


if __name__ == "__main__":
    rng = np.random.default_rng(0)
    x = rng.standard_normal((B, S, D), dtype=np.float32)
    m = np.zeros((B, 1, 1, S), dtype=np.float32)
    W = (rng.standard_normal((D, D), dtype=np.float32) / np.sqrt(D)).astype(np.float32)
    b = np.zeros((D,), dtype=np.float32)
    out = kernel(x, m, W, b)
    print("out", out.shape, out.dtype)


# revision 2
# speedup vs baseline: 1.0676x; 1.0676x over previous
"""BERT self-attention (B=8, S=1024, D=768, H=12) on 8 TRN2 NeuronCores.

Strategy
--------
Data-parallel over batch: core b handles batch element b (no collectives).

Per core, everything is computed in a "transposed" layout that keeps the
tensor engine's contraction dimension in the partition axis:

  1. mixedT[e, s] = sum_d W^T[d, e] * x^T[d, s] + bias[e] in fp32r matmuls
     (full fp32 inputs); the psum evacuation adds the per-partition bias
     and rounds to bf16 for the attention stage.
  2. Q=K=V => the score matrix is symmetric: scores[t, s] = scores[s, t].
     The exp'd score tile in [t, s] layout therefore equals the transposed
     (unnormalized) probability matrix needed as the moving operand of the
     context matmul -- no probability transposes at all.
     scores tile = (MIX chunk)^T @ Z_h where Z_h zero-masks the other head
     of the pair (kills cross-head terms, keeps K=128 partition-aligned
     bf16 matmuls at 1 cycle/column).
     U = exp(0.125 * scores + mask[t]) fused into the ACT psum evacuation
     (mask enters as the per-partition bias), output cast to bf16.
  3. ctx'^T[dh, s] (+ softmax denominator as row 64) accumulate in fp32
     psum over the eight t-chunks with stationary [xl | ones] [128, 65],
     moving U.
  4. PE-transpose of ctx'^T 128-column chunks gives ctx[s, dh] with the
     denominator as column 64; reciprocal + per-partition scalar multiply
     normalizes during the psum evacuation; DMA to the output in natural
     [s, d] layout.
"""

import numpy as np

import concourse.bacc as bacc
import concourse.tile as tile
from concourse import mybir
from concourse.bass_utils import run_bass_kernel_spmd
from concourse.masks import make_identity

B, S, D = 8, 1024, 768
H, DH = 12, 64
NP = 6            # e-tile pairs (2 heads each)
NT = 8            # t-chunks / s-chunks of 128
F32 = mybir.dt.float32
F32R = mybir.dt.float32r
BF16 = mybir.dt.bfloat16
EXP = mybir.ActivationFunctionType.Exp

_CACHED_NC = None


def build_nc():
    nc = bacc.Bacc("TRN2", target_bir_lowering=False)

    xT = nc.dram_tensor("xT", [D, S], F32R, kind="ExternalInput")
    wT = nc.dram_tensor("wT", [D, D], F32R, kind="ExternalInput")
    bias_d = nc.dram_tensor("bias_d", [128, NP], F32, kind="ExternalInput")
    mask_d = nc.dram_tensor("mask_d", [128, NT], F32, kind="ExternalInput")
    out_d = nc.dram_tensor("out", [S, D], F32, kind="ExternalOutput")

    with tile.TileContext(nc) as tc:
        with (
            tc.tile_pool(name="consts", bufs=1) as consts,
            tc.tile_pool(name="big", bufs=1) as big,
            tc.tile_pool(name="upool", bufs=3) as upool,
            tc.tile_pool(name="ctpool", bufs=2) as ctpool,
            tc.tile_pool(name="outpool", bufs=4) as outpool,
            tc.tile_pool(name="rpool", bufs=4) as rpool,
            tc.tile_pool(name="ps_s", bufs=2, space="PSUM") as ps_s,
            tc.tile_pool(name="ps_c", bufs=1, space="PSUM") as ps_c,
            tc.tile_pool(name="ps_t", bufs=2, space="PSUM") as ps_t,
        ):
            ident32 = consts.tile([128, 128], F32)
            make_identity(nc, ident32)
            identbf = consts.tile([128, 128], BF16)
            make_identity(nc, identbf)
            bias_t = consts.tile([128, NP], F32)
            nc.sync.dma_start(out=bias_t, in_=bias_d[:, :])
            mask_t = consts.tile([128, NT], F32)
            nc.sync.dma_start(out=mask_t, in_=mask_d[:, :])

            wts = big.tile([128, NP, D], F32R)
            xts = big.tile([128, NP, S], F32R)
            for k in range(NP):
                nc.sync.dma_start(out=wts[:, k, :], in_=wT[k * 128:(k + 1) * 128, :])
                nc.sync.dma_start(out=xts[:, k, :], in_=xT[k * 128:(k + 1) * 128, :])

            mixbf = big.tile([128, NP, S], BF16)

            # Persistent ping-pong Z and xl_n tiles: the static parts (zero
            # half / ones column) are initialized once, outside the pair loop.
            zt = [[big.tile([128, S], BF16, name=f"z{q}{p}") for p in range(2)]
                  for q in range(2)]
            xlt = [[big.tile([128, NT, DH + 1], BF16, name=f"xl{q}{p}")
                    for p in range(2)] for q in range(2)]
            for q in range(2):
                olo = (1 - q) * 64
                for p in range(2):
                    nc.gpsimd.memset(zt[q][p][olo:olo + 64, :], 0.0)
                    nc.gpsimd.memset(xlt[q][p], 1.0)

            for j in range(NP):
                pp = j % 2  # ping-pong slot
                # ---- mixedT for head pair j (fp32r matmuls) ----
                pm = ps_s.tile([128, S], F32, name="psc")
                for n in range(2):
                    for k in range(NP):
                        nc.tensor.matmul(
                            pm[:, n * 512:(n + 1) * 512],
                            lhsT=wts[:, k, j * 128:(j + 1) * 128],
                            rhs=xts[:, k, n * 512:(n + 1) * 512],
                            start=(k == 0),
                            stop=(k == NP - 1),
                        )
                nc.vector.tensor_scalar_add(mixbf[:, j, :], pm, bias_t[:, j:j + 1])

                # ---- zero-masked copies (one per head of the pair) ----
                zs = []
                for q in range(2):
                    z = zt[q][pp]
                    lo = q * 64
                    nc.vector.tensor_copy(out=z[lo:lo + 64, :],
                                          in_=mixbf[lo:lo + 64, j, :])
                    zs.append(z)

                # ---- xl in natural layout (+ ones column) via PE transpose ----
                xlns = [xlt[0][pp], xlt[1][pp]]
                for i in range(NT):
                    pt = ps_t.tile([128, 128], BF16, name="pt")
                    nc.tensor.transpose(
                        pt, mixbf[:, j, i * 128:(i + 1) * 128], identbf
                    )
                    for q in range(2):
                        nc.vector.tensor_copy(
                            out=xlns[q][:, i, 0:DH], in_=pt[:, q * 64:q * 64 + 64]
                        )

                # ---- attention for the two heads ----
                for q in range(2):
                    h = 2 * j + q
                    pc = ps_c.tile([DH + 1, S], F32, name="pc")
                    for i in range(NT):
                        psc = ps_s.tile([128, S], F32, name="psc")
                        for n in range(2):
                            nc.tensor.matmul(
                                psc[:, n * 512:(n + 1) * 512],
                                lhsT=mixbf[:, j, i * 128:(i + 1) * 128],
                                rhs=zs[q][:, n * 512:(n + 1) * 512],
                                start=True,
                                stop=True,
                            )
                        u = upool.tile([128, S], BF16, name="u")
                        nc.scalar.activation(
                            out=u, in_=psc, func=EXP,
                            bias=mask_t[:, i:i + 1], scale=0.125,
                        )
                        for n in range(2):
                            nc.tensor.matmul(
                                pc[:, n * 512:(n + 1) * 512],
                                lhsT=xlns[q][:, i, :],
                                rhs=u[:, n * 512:(n + 1) * 512],
                                start=(i == 0),
                                stop=(i == NT - 1),
                            )
                    ct = ctpool.tile([DH + 1, S], F32, name="ct")
                    nc.vector.tensor_copy(out=ct, in_=pc)
                    for sj in range(NT):
                        po = ps_t.tile([128, DH + 1], F32, name="pt")
                        nc.tensor.transpose(
                            po,
                            ct[:, sj * 128:(sj + 1) * 128],
                            ident32[0:DH + 1, 0:DH + 1],
                        )
                        rcol = rpool.tile([128, 1], F32, name="rcol")
                        nc.vector.reciprocal(out=rcol, in_=po[:, DH:DH + 1])
                        ot = outpool.tile([128, DH], F32, name="ot")
                        nc.vector.tensor_scalar_mul(ot, po[:, 0:DH], rcol)
                        nc.sync.dma_start(
                            out=out_d[sj * 128:(sj + 1) * 128, h * 64:(h + 1) * 64],
                            in_=ot,
                        )

    nc.compile()
    return nc


def kernel(x, attention_mask, W, b, _profile=None):
    global _CACHED_NC
    if _CACHED_NC is None:
        _CACHED_NC = build_nc()
    nc = _CACHED_NC

    x = np.asarray(x, dtype=np.float32)
    attention_mask = np.asarray(attention_mask, dtype=np.float32)
    W = np.asarray(W, dtype=np.float32)
    b = np.asarray(b, dtype=np.float32)

    wT = np.ascontiguousarray(W.T)
    bias_cols = np.ascontiguousarray(b.reshape(NP, 128).T)

    in_maps = []
    for i in range(B):
        in_maps.append({
            "xT": np.ascontiguousarray(x[i].T),
            "wT": wT,
            "bias_d": bias_cols,
            "mask_d": np.ascontiguousarray(
                attention_mask[i, 0, 0].reshape(NT, 128).T
            ),
        })

    kwargs = dict(_profile) if _profile else {}
    res = run_bass_kernel_spmd(nc, in_maps, core_ids=list(range(B)), **kwargs)
    out = np.stack([res.results[i]["out"] for i in range(B)], axis=0)
    if _profile:
        kernel.last_results = res
    return out


if __name__ == "__main__":
    rng = np.random.default_rng(0)
    x = rng.standard_normal((B, S, D), dtype=np.float32)
    m = np.zeros((B, 1, 1, S), dtype=np.float32)
    W = (rng.standard_normal((D, D), dtype=np.float32) / np.sqrt(D)).astype(np.float32)
    b = np.zeros((D,), dtype=np.float32)
    out = kernel(x, m, W, b)
    print("out", out.shape, out.dtype)
